# revision 2
# baseline (speedup 1.0000x reference)
"""ChildSum TreeLSTM on a complete binary tree — Trainium2 Bass kernel, v4.

Sharding: data-parallel over the batch of trees (B=8 -> 8 NeuronCores, one
tree per core); weights replicated.  On-chip layout is transposed
[feature, node]; the host pre-transposes x per core.

Key structure (ACT is the bottleneck engine; everything serves it):
- Depth-split precision: levels >= L0 (=12, 87% of nodes) run fp8e4
  DoubleRow matmuls (full 256-deep contraction per instruction, 2x PE rate);
  shallow levels run bf16.  fp8 quantization error injected at depth d is
  attenuated ~0.5x per level by the forget gates, so the deep-fp8 error
  contribution at the root is small (measured ~7e-3 vs the 2e-2 gate).
- Biases enter PSUM via tiny selector matmuls (start=True), so activation
  instructions span whole multi-bank PSUM tiles: one sigmoid over [i|o]
  (4C cols), one tanh(u), one sigmoid over both f banks.
- Elementwise in bf16 with unit-stride innermost dims (DVE 2x mode);
  child-pair reductions and h-writes on the Pool engine; root in fp32.
- Chunk emission order keeps >=1 unrelated chunk between children and
  parent; tanh(c)/h are deferred one slot; child-sums are hoisted one slot
  ahead so recurrent matmuls never wait on the h/c chain.
"""

import sys

sys.path.insert(0, "/opt/trn_rl_repo")

import numpy as np
import ml_dtypes

import bass_rust
import concourse.bass as bass
import concourse.mybir as mybir
from concourse.tile import TileContext
from concourse.bass_utils import run_bass_kernel_spmd

P = 128
D_IN = 256
D_H = 256
FP8 = mybir.dt.float8e4
BF16 = mybir.dt.bfloat16
F32 = mybir.dt.float32
AF = mybir.ActivationFunctionType
ALU = mybir.AluOpType
DR = mybir.MatmulPerfMode.DoubleRow
L0 = 12  # levels >= L0 use fp8 DoubleRow matmuls

_NC_CACHE = {}


def _split_waits(nc, compute_limit=1, dma_limit=1):
    """Walrus in this container accepts few fused sync-waits per instruction
    (1 for DMA descriptors, ~2 for compute).  Move excess waits onto
    same-engine nop instructions inserted right before the offender."""
    eng_map = {
        mybir.EngineType.DVE: nc.vector,
        mybir.EngineType.Activation: nc.scalar,
        mybir.EngineType.PE: nc.tensor,
        mybir.EngineType.Pool: nc.gpsimd,
        mybir.EngineType.SP: nc.sync,
    }

    active_block = [None]

    def make_nop(engine):
        bi = eng_map[engine].nop()
        inst = bi.ins
        ab = active_block[0]
        if ab is not None and ab.instructions and ab.instructions[-1] is inst:
            ab.instructions.pop()
            return inst
        for f in nc.m.functions:
            for b in f.blocks:
                if b.instructions and b.instructions[-1] is inst:
                    b.instructions.pop()
                    active_block[0] = b
                    return inst
        raise RuntimeError("nop not found")

    dma_types = {"InstDMACopy", "InstDMA", "InstDmaTransposeAnt", "InstDrain"}
    for f in nc.m.functions:
        for b in f.blocks:
            new = []
            for inst in list(b.instructions):
                si = inst.sync_info
                waits = list(si.on_wait) if si is not None and si.on_wait else []
                tname = type(inst).__name__
                eng = getattr(inst, "engine", None)
                limit = dma_limit if tname in dma_types else compute_limit
                nop_limit = dma_limit if eng == mybir.EngineType.SP else compute_limit
                if len(waits) > limit and eng in eng_map:
                    excess, keep = waits[:-limit] if limit else waits, waits[-limit:] if limit else []
                    for i0 in range(0, len(excess), nop_limit):
                        nop = make_nop(eng)
                        nop.sync_info = bass_rust.SyncInfo(
                            on_wait=excess[i0:i0 + nop_limit], on_update=[]
                        )
                        new.append(nop)
                    inst.sync_info = bass_rust.SyncInfo(
                        on_wait=keep, on_update=list(si.on_update) if si.on_update else []
                    )
                new.append(inst)
            b.instructions.clear()
            b.instructions.extend(new)


def build_nc(L, trace_sim=False, debug=False):
    """Single-core SPMD Bass program for a tree with L leaves."""
    D = int(np.log2(L))
    assert 2**D == L
    N = 2 * L - 1
    n_top = min(2**L0 - 1, N)      # nodes in bf16 levels (heap cols [0, n_top))

    nc = bass.Bass()

    xT8 = nc.dram_tensor("xT8", [D_IN, max(N - n_top, 1)], FP8, kind="ExternalInput")
    w8_iou_d = nc.dram_tensor("w8_iou", [P, 2 * 3 * D_H], FP8, kind="ExternalInput")
    u8_iou_d = nc.dram_tensor("u8_iou", [P, 2 * 3 * D_H], FP8, kind="ExternalInput")
    w8_f_d = nc.dram_tensor("w8_f", [P, 2 * D_H], FP8, kind="ExternalInput")
    u8_f_d = nc.dram_tensor("u8_f", [P, 2 * D_H], FP8, kind="ExternalInput")
    if n_top:
        xT16 = nc.dram_tensor("xT16", [D_IN, n_top], BF16, kind="ExternalInput")
        wb_iou_d = nc.dram_tensor("wb_iou", [P, 2 * 3 * D_H], BF16, kind="ExternalInput")
        ub_iou_d = nc.dram_tensor("ub_iou", [P, 2 * 3 * D_H], BF16, kind="ExternalInput")
        wb_f_d = nc.dram_tensor("wb_f", [P, 2 * D_H], BF16, kind="ExternalInput")
        ub_f_d = nc.dram_tensor("ub_f", [P, 2 * D_H], BF16, kind="ExternalInput")
    b_io_d = nc.dram_tensor("b_io_t", [2, 2 * P], BF16, kind="ExternalInput")
    b_io4_d = nc.dram_tensor("b_io4_t", [4, P], BF16, kind="ExternalInput")
    b_u_d = nc.dram_tensor("b_u_t", [2, P], BF16, kind="ExternalInput")
    b_f_d = nc.dram_tensor("b_f_t", [1, 2 * P], BF16, kind="ExternalInput")
    b_f2_d = nc.dram_tensor("b_f2_t", [2, P], BF16, kind="ExternalInput")
    sel2_d = nc.dram_tensor("sel2", [2, 512], BF16, kind="ExternalInput")
    sel4_d = nc.dram_tensor("sel4", [4, 512], BF16, kind="ExternalInput")
    ones_d = nc.dram_tensor("ones", [1, 512], BF16, kind="ExternalInput")
    out_d = nc.dram_tensor("out", [2, D_H], F32, kind="ExternalOutput")

    C_LEAF = min(256, L)

    def n_chunks(lvl):
        n = 2**lvl
        if lvl == D:
            return L // C_LEAF
        return n // min(n, 256)

    with TileContext(nc, trace_sim=trace_sim) as tc:
        with (
            tc.tile_pool(name="const", bufs=1) as cpool,
            tc.tile_pool(name="xa", bufs=6) as xpool,
            tc.tile_pool(name="h", bufs=10) as hpool,
            tc.tile_pool(name="c", bufs=10) as cfpool,
            tc.tile_pool(name="g", bufs=4) as gpool,
            tc.tile_pool(name="psio", bufs=2, space="PSUM") as psio_pool,
            tc.tile_pool(name="psu", bufs=2, space="PSUM") as psu_pool,
            tc.tile_pool(name="psf", bufs=1, space="PSUM") as psf_pool,
        ):
            # ---- replicated weights / biases / selectors into SBUF ----
            w8_iou = cpool.tile([P, 2 * 3 * D_H], FP8, tag="w8_iou", name="w8_iou")
            u8_iou = cpool.tile([P, 2 * 3 * D_H], FP8, tag="u8_iou", name="u8_iou")
            w8_f = cpool.tile([P, 2 * D_H], FP8, tag="w8_f", name="w8_f")
            u8_f = cpool.tile([P, 2 * D_H], FP8, tag="u8_f", name="u8_f")
            b_io = cpool.tile([2, 2 * P], BF16, tag="b_io", name="b_io")
            b_io4 = cpool.tile([4, P], BF16, tag="b_io4", name="b_io4")
            b_u = cpool.tile([2, P], BF16, tag="b_u", name="b_u")
            b_f = cpool.tile([1, 2 * P], BF16, tag="b_f", name="b_f")
            b_f2 = cpool.tile([2, P], BF16, tag="b_f2", name="b_f2")
            sel2 = cpool.tile([2, 512], BF16, tag="sel2", name="sel2")
            sel4 = cpool.tile([4, 512], BF16, tag="sel4", name="sel4")
            ones = cpool.tile([1, 512], BF16, tag="ones", name="ones")
            loads = [
                (w8_iou, w8_iou_d), (u8_iou, u8_iou_d),
                (w8_f, w8_f_d), (u8_f, u8_f_d),
                (b_io, b_io_d), (b_io4, b_io4_d), (b_u, b_u_d),
                (b_f, b_f_d), (b_f2, b_f2_d),
                (sel2, sel2_d), (sel4, sel4_d), (ones, ones_d),
            ]
            if n_top:
                wb_iou = cpool.tile([P, 2 * 3 * D_H], BF16, tag="wb_iou", name="wb_iou")
                ub_iou = cpool.tile([P, 2 * 3 * D_H], BF16, tag="ub_iou", name="ub_iou")
                wb_f = cpool.tile([P, 2 * D_H], BF16, tag="wb_f", name="wb_f")
                ub_f = cpool.tile([P, 2 * D_H], BF16, tag="ub_f", name="ub_f")
                loads += [
                    (wb_iou, wb_iou_d), (ub_iou, ub_iou_d),
                    (wb_f, wb_f_d), (ub_f, ub_f_d),
                ]
            for t, d in loads:
                nc.gpsimd.dma_start(out=t, in_=d[:, :])

            w8_iou3 = w8_iou.rearrange("p (kt o) -> p kt o", kt=2)
            u8_iou3 = u8_iou.rearrange("p (kt o) -> p kt o", kt=2)
            w8_f3 = w8_f.rearrange("p (kt o) -> p kt o", kt=2)
            u8_f3 = u8_f.rearrange("p (kt o) -> p kt o", kt=2)
            if n_top:
                wb_iou3 = wb_iou.rearrange("p (kt o) -> p kt o", kt=2)
                ub_iou3 = ub_iou.rearrange("p (kt o) -> p kt o", kt=2)
                wb_f3 = wb_f.rearrange("p (kt o) -> p kt o", kt=2)
                ub_f3 = ub_f.rearrange("p (kt o) -> p kt o", kt=2)

            h_tiles = {}  # (lvl, ti) -> tile [128, 2*S]
            c_tiles = {}
            h_dbg = {}
            c_dbg = {}
            g_dbg = {}

            def deep(lvl):
                return lvl >= L0

            def h_dtype(lvl):
                # h of level l is the matmul rhs of level l-1
                if lvl == 0:
                    return F32
                return FP8 if deep(lvl - 1) else BF16

            def load_x(lvl, col0, C):
                off = 2**lvl - 1
                dt = FP8 if deep(lvl) else BF16
                xa = xpool.tile([P, 2 * C], dt, tag="xa", name="xa")
                src = xT8 if deep(lvl) else xT16
                soff = off - n_top if deep(lvl) else off
                for kt in range(2):
                    nc.sync.dma_start(
                        out=xa[:, kt * C:(kt + 1) * C],
                        in_=src[kt * P:(kt + 1) * P, soff + col0: soff + col0 + C],
                    )
                return xa

            def alloc_hc(lvl, j, C):
                n = 2**lvl
                S = min(n, 512)
                ti, co = (j * C) // S, (j * C) % S
                if co == 0:
                    h_tiles[(lvl, ti)] = hpool.tile(
                        [P, 2 * S], h_dtype(lvl), tag="h", name="h"
                    )
                    c_tiles[(lvl, ti)] = cfpool.tile(
                        [P, 2 * S], F32 if lvl == 0 else BF16, tag="c", name="c"
                    )
                if debug:
                    h_dbg[(lvl, ti)] = h_tiles[(lvl, ti)]
                    c_dbg[(lvl, ti)] = c_tiles[(lvl, ti)]
                h3 = h_tiles[(lvl, ti)].rearrange("p (kt s) -> p kt s", kt=2)
                c3 = c_tiles[(lvl, ti)].rearrange("p (kt s) -> p kt s", kt=2)
                return h3[:, :, co:co + C], c3[:, :, co:co + C]

            def sel_rhs(base, base_span, span):
                """Selector rhs whose row q is an indicator of [q*span, (q+1)*span)."""
                nrow = base.shape[0]
                m = base_span // span
                if m == 1:
                    return base[:, : nrow * span]
                return base[:, ::m][:, : nrow * span]

            hs_map = {}

            def emit_hs(lvl, j):
                """Child-sum of h for an internal chunk; emitted one slot ahead
                of the chunk so its recurrent matmuls never wait."""
                n = 2**lvl
                C = min(n, 256)
                hch3 = h_tiles[(lvl + 1, j)].rearrange("p (kt s) -> p kt s", kt=2)
                hs = gpool.tile([P, 2 * C], FP8 if deep(lvl) else BF16,
                                tag="hs", name="hs")
                hs3 = hs.rearrange("p (kt c) -> p kt c", kt=2)
                nc.gpsimd.tensor_tensor(
                    hs3, hch3[:, :, 0::2], hch3[:, :, 1::2], ALU.add
                )
                hs_map[(lvl, j)] = hs

            def emit_A(lvl, j):
                is_leaf = lvl == D
                is_deep = deep(lvl)
                n = 2**lvl
                C = C_LEAF if is_leaf else min(n, 256)
                xa = load_x(lvl, j * C, C)
                h_sl, c_sl = alloc_hc(lvl, j, C)

                ps_io = psio_pool.tile([P, 4 * C], F32, tag="psio", name="psio")
                ps_u = psu_pool.tile([P, 2 * C], F32, tag="psu", name="psu")

                if not is_leaf:
                    hch = h_tiles[(lvl + 1, j)]
                    cch = c_tiles[(lvl + 1, j)]
                    hch3 = hch.rearrange("p (kt s) -> p kt s", kt=2)  # [128,2,2C]
                    cch3 = cch.rearrange("p (kt s) -> p kt s", kt=2)
                    if (lvl, j) not in hs_map:
                        emit_hs(lvl, j)
                    hs = hs_map.pop((lvl, j))
                    ps_f = psf_pool.tile([P, 4 * C], F32, tag="psf", name="psf")

                # --- bias fills (selector matmuls, start=True zeroes PSUM) ---
                # start=True zeroes the WHOLE 2KB bank (zero-region
                # granularity), so emit exactly one start matmul per bank.
                if 4 * C <= 512:
                    nc.tensor.matmul(
                        out=ps_io, lhsT=b_io4, rhs=sel_rhs(sel4, 128, C),
                        start=True, stop=False, skip_group_check=True,
                    )
                else:
                    for bk in range(2):
                        nc.tensor.matmul(
                            out=ps_io[:, bk * 2 * C:(bk + 1) * 2 * C],
                            lhsT=b_io[:, bk * P:(bk + 1) * P],
                            rhs=sel_rhs(sel2, 256, C),
                            start=True, stop=False, skip_group_check=True,
                        )
                nc.tensor.matmul(
                    out=ps_u, lhsT=b_u, rhs=sel_rhs(sel2, 256, C),
                    start=True, stop=False, skip_group_check=True,
                )
                if not is_leaf:
                    if 4 * C <= 512:
                        nc.tensor.matmul(
                            out=ps_f, lhsT=b_f2, rhs=sel_rhs(sel2, 256, 2 * C),
                            start=True, stop=False, skip_group_check=True,
                        )
                    else:
                        for k in range(2):
                            nc.tensor.matmul(
                                out=ps_f[:, k * 2 * C:(k + 1) * 2 * C],
                                lhsT=b_f[:, k * P:(k + 1) * P],
                                rhs=ones[:, :2 * C],
                                start=True, stop=False, skip_group_check=True,
                            )

                # --- feed-forward fills: W x ---
                if is_deep:
                    rhs_x = xa.rearrange("p (kt c) -> p kt c", kt=2)
                    for q in range(4):
                        nc.tensor.matmul(
                            out=ps_io[:, q * C:(q + 1) * C],
                            lhsT=w8_iou3[:, :, q * P:(q + 1) * P], rhs=rhs_x,
                            perf_mode=DR, start=False, stop=is_leaf,
                            skip_group_check=True,
                        )
                    for k in range(2):
                        o0 = 4 * P + k * P
                        nc.tensor.matmul(
                            out=ps_u[:, k * C:(k + 1) * C],
                            lhsT=w8_iou3[:, :, o0:o0 + P], rhs=rhs_x,
                            perf_mode=DR, start=False, stop=is_leaf,
                            skip_group_check=True,
                        )
                    if not is_leaf:
                        xb = xa.rearrange(
                            "p (kt c one) -> p kt c one", kt=2, one=1
                        ).broadcast_to((P, 2, C, 2))
                        for k in range(2):
                            nc.tensor.matmul(
                                out=ps_f[:, k * 2 * C:(k + 1) * 2 * C],
                                lhsT=w8_f3[:, :, k * P:(k + 1) * P], rhs=xb,
                                perf_mode=DR, start=False, stop=False,
                                skip_group_check=True,
                            )
                else:
                    for q in range(4):
                        for kt in range(2):
                            nc.tensor.matmul(
                                out=ps_io[:, q * C:(q + 1) * C],
                                lhsT=wb_iou3[:, kt, q * P:(q + 1) * P],
                                rhs=xa[:, kt * C:(kt + 1) * C],
                                start=False, stop=is_leaf and kt == 1,
                                skip_group_check=True,
                            )
                    for k in range(2):
                        o0 = 4 * P + k * P
                        for kt in range(2):
                            nc.tensor.matmul(
                                out=ps_u[:, k * C:(k + 1) * C],
                                lhsT=wb_iou3[:, kt, o0:o0 + P],
                                rhs=xa[:, kt * C:(kt + 1) * C],
                                start=False, stop=is_leaf and kt == 1,
                                skip_group_check=True,
                            )
                    if not is_leaf:
                        for k in range(2):
                            for kt in range(2):
                                xb = xa[:, kt * C:(kt + 1) * C].rearrange(
                                    "p (c one) -> p c one", one=1
                                ).broadcast_to((P, C, 2))
                                nc.tensor.matmul(
                                    out=ps_f[:, k * 2 * C:(k + 1) * 2 * C],
                                    lhsT=wb_f3[:, kt, k * P:(k + 1) * P], rhs=xb,
                                    start=False, stop=False,
                                    skip_group_check=True,
                                )

                # --- recurrent fills: U h ---
                if not is_leaf:
                    if is_deep:
                        rhs_h = hs.rearrange("p (kt c) -> p kt c", kt=2)
                        for q in range(4):
                            nc.tensor.matmul(
                                out=ps_io[:, q * C:(q + 1) * C],
                                lhsT=u8_iou3[:, :, q * P:(q + 1) * P], rhs=rhs_h,
                                perf_mode=DR, start=False, stop=True,
                                skip_group_check=True,
                            )
                        for k in range(2):
                            o0 = 4 * P + k * P
                            nc.tensor.matmul(
                                out=ps_u[:, k * C:(k + 1) * C],
                                lhsT=u8_iou3[:, :, o0:o0 + P], rhs=rhs_h,
                                perf_mode=DR, start=False, stop=True,
                                skip_group_check=True,
                            )
                        for k in range(2):
                            nc.tensor.matmul(
                                out=ps_f[:, k * 2 * C:(k + 1) * 2 * C],
                                lhsT=u8_f3[:, :, k * P:(k + 1) * P], rhs=hch3,
                                perf_mode=DR, start=False, stop=True,
                                skip_group_check=True,
                            )
                    else:
                        for q in range(4):
                            for kt in range(2):
                                nc.tensor.matmul(
                                    out=ps_io[:, q * C:(q + 1) * C],
                                    lhsT=ub_iou3[:, kt, q * P:(q + 1) * P],
                                    rhs=hs[:, kt * C:(kt + 1) * C],
                                    start=False, stop=kt == 1,
                                    skip_group_check=True,
                                )
                        for k in range(2):
                            o0 = 4 * P + k * P
                            for kt in range(2):
                                nc.tensor.matmul(
                                    out=ps_u[:, k * C:(k + 1) * C],
                                    lhsT=ub_iou3[:, kt, o0:o0 + P],
                                    rhs=hs[:, kt * C:(kt + 1) * C],
                                    start=False, stop=kt == 1,
                                    skip_group_check=True,
                                )
                        for k in range(2):
                            for kt in range(2):
                                nc.tensor.matmul(
                                    out=ps_f[:, k * 2 * C:(k + 1) * 2 * C],
                                    lhsT=ub_f3[:, kt, k * P:(k + 1) * P],
                                    rhs=hch3[:, kt, :],
                                    start=False, stop=kt == 1,
                                    skip_group_check=True,
                                )

                # --- activations (merged, PSUM-sourced) ---
                sio = gpool.tile([P, 4 * C], BF16, tag="sio", name="sio")
                tu = gpool.tile([P, 2 * C], BF16, tag="tu", name="tu")
                if debug and lvl == D and j == 0:
                    g_dbg["sio"] = sio
                    g_dbg["tu"] = tu
                if is_leaf:
                    nc.scalar.activation(sio, ps_io, AF.Sigmoid)
                    nc.scalar.activation(tu, ps_u, AF.Tanh)
                    nc.vector.tensor_tensor(c_sl, sio[:, :2 * C], tu, ALU.mult)
                else:
                    sf = gpool.tile([P, 4 * C], BF16, tag="sf", name="sf")
                    nc.scalar.activation(sf, ps_f, AF.Sigmoid)
                    nc.scalar.activation(sio, ps_io, AF.Sigmoid)
                    nc.scalar.activation(tu, ps_u, AF.Tanh)
                    sf3 = sf.rearrange("p (kt c) -> p kt c", kt=2)  # [128,2,2C]
                    fc = gpool.tile([P, 4 * C], BF16, tag="fc", name="fc")
                    fc3 = fc.rearrange("p (kt c) -> p kt c", kt=2)
                    nc.vector.tensor_tensor(fc3, sf3, cch3, ALU.mult)
                    cf = gpool.tile([P, 2 * C], BF16, tag="cf", name="cf")
                    cf3 = cf.rearrange("p (kt c) -> p kt c", kt=2)
                    nc.vector.tensor_tensor(
                        cf3, fc3[:, :, 0::2], fc3[:, :, 1::2], ALU.add
                    )
                    iu = gpool.tile([P, 2 * C], BF16, tag="iu", name="iu")
                    nc.vector.tensor_tensor(iu, sio[:, :2 * C], tu, ALU.mult)
                    nc.vector.tensor_tensor(c_sl, iu, cf, ALU.add)
                return sio, c_sl, h_sl, C

            def emit_B(state):
                """tanh(c) and h; deferred one slot to keep the activation
                queue from stalling on the c-update chain."""
                sio, c_sl, h_sl, C = state
                tc_t = gpool.tile([P, 2 * C], BF16, tag="tc", name="tc")
                nc.scalar.activation(tc_t, c_sl, AF.Tanh)
                nc.gpsimd.tensor_tensor(h_sl, sio[:, 2 * C:4 * C], tc_t, ALU.mult)

            def gen_order():
                """Post-order, but with at least one unrelated chunk between
                any chunk and its parent."""
                parent_of = {}
                remaining = {}
                for lvl in range(D):
                    r2 = n_chunks(lvl + 1) // n_chunks(lvl) == 2
                    for j in range(n_chunks(lvl)):
                        kids = ([(lvl + 1, 2 * j), (lvl + 1, 2 * j + 1)]
                                if r2 else [(lvl + 1, j)])
                        for k in kids:
                            parent_of[k] = (lvl, j)
                        remaining[(lvl, j)] = len(kids)
                order = []
                last_child_at = {}
                ready = []

                def mark(ch):
                    order.append(ch)
                    p = parent_of.get(ch)
                    if p is not None:
                        remaining[p] -= 1
                        last_child_at[p] = len(order) - 1
                        if remaining[p] == 0:
                            ready.append(p)

                li, nleaf = 0, n_chunks(D)
                while li < nleaf or ready:
                    if ready and last_child_at[ready[0]] < len(order) - 1:
                        mark(ready.pop(0))
                    elif li < nleaf:
                        mark((D, li))
                        li += 1
                    else:
                        mark(ready.pop(0))
                return order

            order = gen_order()
            children = {}
            for ch in order:
                lvl, j = ch
                if lvl < D:
                    r2 = n_chunks(lvl + 1) // n_chunks(lvl) == 2
                    children[ch] = set(
                        [(lvl + 1, 2 * j), (lvl + 1, 2 * j + 1)] if r2
                        else [(lvl + 1, j)]
                    )

            b_done = set()
            hs_emitted = set()

            def hoist_hs(cur_idx):
                for k in range(cur_idx + 1, min(cur_idx + 5, len(order))):
                    ch2 = order[k]
                    if ch2 in children and ch2 not in hs_emitted \
                            and children[ch2] <= b_done:
                        emit_hs(*ch2)
                        hs_emitted.add(ch2)

            prev_state = None
            prev_ch = None
            for idx, ch in enumerate(order):
                if prev_state is not None and prev_ch in children.get(ch, ()):
                    emit_B(prev_state)
                    b_done.add(prev_ch)
                    prev_state = None
                    hoist_hs(idx - 1)
                state = emit_A(*ch)
                if prev_state is not None:
                    emit_B(prev_state)
                    b_done.add(prev_ch)
                prev_state, prev_ch = state, ch
                hoist_hs(idx)
            emit_B(prev_state)

            # root h (f32) and c -> out
            h3 = h_tiles[(0, 0)].rearrange("p (kt s) -> p kt s", kt=2)
            c3 = c_tiles[(0, 0)].rearrange("p (kt s) -> p kt s", kt=2)
            for kt in range(2):
                nc.sync.dma_start(
                    out=out_d[0:1, kt * P:(kt + 1) * P], in_=h3[:, kt, 0:1]
                )
                nc.sync.dma_start(
                    out=out_d[1:2, kt * P:(kt + 1) * P], in_=c3[:, kt, 0:1]
                )
            if debug:
                for nm, t in list(g_dbg.items()):
                    d = nc.dram_tensor(f"dbg_{nm}", list(t.shape),
                                       t.dtype, kind="ExternalOutput")
                    nc.sync.dma_start(out=d[:, :], in_=t)
                for (lvl, ti), t in list(h_dbg.items()):
                    d = nc.dram_tensor(f"dbg_h_{lvl}_{ti}", list(t.shape),
                                       t.dtype, kind="ExternalOutput")
                    nc.sync.dma_start(out=d[:, :], in_=t)
                for (lvl, ti), t in list(c_dbg.items()):
                    d = nc.dram_tensor(f"dbg_c_{lvl}_{ti}", list(t.shape),
                                       t.dtype, kind="ExternalOutput")
                    nc.sync.dma_start(out=d[:, :], in_=t)

    _split_waits(nc)
    return nc


def get_nc(L):
    if L not in _NC_CACHE:
        _NC_CACHE[L] = build_nc(L)
    return _NC_CACHE[L]


def _pack_w(w, out_dim, np_dtype):
    """[256, out] fp32 -> [128, 2*out] with kt-major columns."""
    w = np.asarray(w, dtype=np.float32).reshape(2, P, out_dim)
    return np.ascontiguousarray(
        w.transpose(1, 0, 2).reshape(P, 2 * out_dim)
    ).astype(np_dtype)


def prepare_in_maps(x, W_iou, b_iou, U_iou, W_f, b_f, U_f):
    fp8 = ml_dtypes.float8_e4m3
    bf16 = ml_dtypes.bfloat16
    B, N, _ = x.shape
    L = (N + 1) // 2
    D = int(np.log2(L))
    n_top = min(2**L0 - 1, N)
    b_iou = np.asarray(b_iou, dtype=np.float32)
    b_f_v = np.asarray(b_f, dtype=np.float32)
    sel2 = np.zeros((2, 512), dtype=bf16)
    sel2[0, :256] = 1
    sel2[1, 256:] = 1
    sel4 = np.zeros((4, 512), dtype=bf16)
    for q in range(4):
        sel4[q, q * 128:(q + 1) * 128] = 1
    ones = np.ones((1, 512), dtype=bf16)
    common = {
        "w8_iou": _pack_w(W_iou, 3 * D_H, fp8),
        "u8_iou": _pack_w(U_iou, 3 * D_H, fp8),
        "w8_f": _pack_w(W_f, D_H, fp8),
        "u8_f": _pack_w(U_f, D_H, fp8),
        "b_io_t": np.ascontiguousarray(
            b_iou[:512].reshape(2, 2, P).transpose(1, 0, 2).reshape(2, 2 * P)
        ).astype(bf16),
        "b_io4_t": np.ascontiguousarray(b_iou[:512].reshape(4, P)).astype(bf16),
        "b_u_t": np.ascontiguousarray(b_iou[512:].reshape(2, P)).astype(bf16),
        "b_f_t": np.ascontiguousarray(b_f_v.reshape(1, 2 * P)).astype(bf16),
        "b_f2_t": np.ascontiguousarray(b_f_v.reshape(2, P)).astype(bf16),
        "sel2": sel2,
        "sel4": sel4,
        "ones": ones,
    }
    if n_top:
        common.update({
            "wb_iou": _pack_w(W_iou, 3 * D_H, bf16),
            "ub_iou": _pack_w(U_iou, 3 * D_H, bf16),
            "wb_f": _pack_w(W_f, D_H, bf16),
            "ub_f": _pack_w(U_f, D_H, bf16),
        })
    in_maps = []
    for b in range(B):
        xTb = np.ascontiguousarray(np.asarray(x[b], dtype=np.float32).T)
        x8 = xTb[:, n_top:].astype(fp8)
        if x8.shape[1] == 0:
            x8 = np.zeros((D_IN, 1), dtype=fp8)
        m = {"xT8": x8, **common}
        if n_top:
            m["xT16"] = xTb[:, :n_top].astype(bf16)
        in_maps.append(m)
    return in_maps


def run(inputs, trace=False):
    x = np.asarray(inputs["x"])
    B, N, _ = x.shape
    L = (N + 1) // 2
    nc = get_nc(L)
    in_maps = prepare_in_maps(
        x, inputs["W_iou"], inputs["b_iou"], inputs["U_iou"],
        inputs["W_f"], inputs["b_f"], inputs["U_f"],
    )
    res = run_bass_kernel_spmd(nc, in_maps, core_ids=list(range(B)), trace=trace)
    out = np.zeros((B, 2 * D_H), dtype=np.float32)
    for b in range(B):
        o = np.asarray(res.results[b]["out"], dtype=np.float32)
        out[b, :D_H] = o[0]
        out[b, D_H:] = o[1]
    return out, res


def kernel(**inputs):
    out, _ = run(inputs, trace=False)
    return out
